# revision 1
# baseline (speedup 1.0000x reference)
import numpy as np

B, L, H, D = 2, 65536, 8, 32
BS = 128
NB = L // BS
NCORES = 8
CORES_PER_B = NCORES // B
NB_LOC = NB // CORES_PER_B
CB = 8
NCHUNK = NB_LOC // CB
HD = H * D
D1 = D + 1
SCALE = float(1.0 / np.sqrt(D))

_cached_nc = None


def _build(num_devices=NCORES):
    import concourse.bass as bass
    import concourse.bacc as bacc
    import concourse.tile as tile
    from concourse import mybir
    from contextlib import ExitStack

    f32 = mybir.dt.float32
    bf16 = mybir.dt.bfloat16

    nc = bacc.Bacc(
        "TRN2", target_bir_lowering=False, debug=False, num_devices=num_devices
    )
    QTd = nc.dram_tensor("qt", [NB_LOC, BS, HD], bf16, kind="ExternalInput").ap()
    KTd = nc.dram_tensor("kt", [NB_LOC, BS, HD], bf16, kind="ExternalInput").ap()
    Vd = nc.dram_tensor("v", [NB_LOC, BS, H * D1], bf16, kind="ExternalInput").ap()
    QBd = nc.dram_tensor("qbias", [BS, NB_LOC], f32, kind="ExternalInput").ap()
    Od = nc.dram_tensor("o", [NB_LOC, BS, HD], f32, kind="ExternalOutput").ap()

    with tile.TileContext(nc) as tc, ExitStack() as ctx:
        singles = ctx.enter_context(tc.tile_pool(name="singles", bufs=1))
        qk_pool = ctx.enter_context(tc.tile_pool(name="qk", bufs=3))
        v_pool = ctx.enter_context(tc.tile_pool(name="vp", bufs=3))
        out_pool = ctx.enter_context(tc.tile_pool(name="outp", bufs=3))
        exps_pool = ctx.enter_context(tc.tile_pool(name="exps", bufs=4))
        small_pool = ctx.enter_context(tc.tile_pool(name="small", bufs=12))
        ps_pool = ctx.enter_context(tc.tile_pool(name="ps", bufs=1, space="PSUM"))

        qbias = singles.tile([BS, NB_LOC], f32)
        nc.sync.dma_start(out=qbias, in_=QBd)

        s_ps = ps_pool.tile([BS, 8, 512], f32)

        chunk_tiles = {}

        def ensure_chunk(c):
            if c in chunk_tiles or c >= NCHUNK:
                return
            n0 = c * CB
            qt = qk_pool.tile([BS, CB, HD], bf16, tag="qt")
            nc.sync.dma_start(
                out=qt, in_=QTd[n0 : n0 + CB].rearrange("n p d -> p n d")
            )
            kt = qk_pool.tile([BS, CB, HD], bf16, tag="kt")
            nc.sync.dma_start(
                out=kt, in_=KTd[n0 : n0 + CB].rearrange("n p d -> p n d")
            )
            v_t = v_pool.tile([BS, CB, H, D1], bf16)
            nc.sync.dma_start(
                out=v_t,
                in_=Vd[n0 : n0 + CB].rearrange("n p (h e) -> p n h e", h=H),
            )
            o_sb = out_pool.tile([BS, CB, HD], f32)
            chunk_tiles[c] = (qt, kt, v_t, o_sb)

        def emit_s(n):
            qt, kt, _, _ = chunk_tiles[n // CB]
            j = n % CB
            base = (n % 2) * 4
            for h in range(H):
                hh, r = divmod(h, 4)
                nc.tensor.matmul(
                    s_ps[:, base + r, hh * BS : (hh + 1) * BS],
                    kt[32 * r : 32 * (r + 1), j, hh * BS : (hh + 1) * BS],
                    qt[32 * r : 32 * (r + 1), j, hh * BS : (hh + 1) * BS],
                    start=True,
                    stop=True,
                    tile_position=(32 * r, 0),
                )

        ensure_chunk(0)
        emit_s(0)
        for n in range(NB_LOC):
            ensure_chunk((n + 1) // CB)
            if n + 1 < NB_LOC:
                emit_s(n + 1)

            _, _, v_t, o_sb = chunk_tiles[n // CB]
            j = n % CB
            base = (n % 2) * 4

            exps = exps_pool.tile([BS, 4, 2, BS], bf16)
            nc.scalar.activation(
                exps,
                s_ps[:, base : base + 4, 0 : 2 * BS].rearrange(
                    "p r (a q) -> p r a q", a=2
                ),
                mybir.ActivationFunctionType.Exp,
                scale=SCALE,
            )

            corner = s_ps[:, base : base + 2, 2 * BS : 2 * BS + 4 * D1].rearrange(
                "p b (i x) -> p b i x", i=4
            )
            oc = small_pool.tile([BS, 2, 4, D1], f32, tag="oc")
            for b in range(2):
                for i in range(4):
                    h = b * 4 + i
                    hh, r = divmod(h, 4)
                    c0 = 2 * BS + i * D1
                    nc.tensor.matmul(
                        s_ps[:, base + b, c0 : c0 + D1],
                        exps[:, r, hh],
                        v_t[:, j, h],
                        start=True,
                        stop=True,
                    )
                nc.vector.tensor_copy(oc[:, b : b + 1], corner[:, b : b + 1])

            rs = small_pool.tile([BS, 2, 4, 1], f32, tag="rs")
            nc.gpsimd.tensor_scalar_add(
                rs, oc[:, :, :, D : D + 1], qbias[:, n : n + 1]
            )
            recip = small_pool.tile([BS, 2, 4, 1], f32, tag="recip")
            nc.vector.reciprocal(recip, rs)
            rb = bass.AP(
                tensor=recip.tensor,
                offset=recip.offset,
                ap=[recip.ap[0], recip.ap[1], recip.ap[2], [0, D]],
            )
            nc.gpsimd.tensor_tensor(
                out=o_sb[:, j].rearrange("p (b i e) -> p b i e", b=2, i=4),
                in0=oc[:, :, :, 0:D],
                in1=rb,
                op=mybir.AluOpType.mult,
            )

            if j == CB - 1:
                n0 = (n // CB) * CB
                nc.sync.dma_start(
                    out=Od[n0 : n0 + CB].rearrange("n p d -> p n d"), in_=o_sb
                )

    nc.compile()
    return nc


def _host_prep(Q, K, V, scope_buckets):
    import ml_dtypes

    bf = ml_dtypes.bfloat16
    scope_buckets = np.asarray(scope_buckets)
    starts = scope_buckets[..., 0].astype(np.int64)
    ends = scope_buckets[..., 1].astype(np.int64)
    abs_pos = (np.arange(NB, dtype=np.int64) * BS)[:, None] + np.arange(BS)[None, :]
    valid = (abs_pos[None] >= starts[..., None]) & (abs_pos[None] < ends[..., None])
    valid = valid.astype(np.float32)
    qbias = np.where(valid > 0, np.float32(1e-30), np.float32(1e30)).astype(
        np.float32
    )

    def bucket_T(x):
        xb = np.ascontiguousarray(x).astype(bf).reshape(B, NB, BS, 2, BS)
        xt = xb.transpose(0, 1, 4, 3, 2).reshape(B, NB, BS, HD)
        return np.ascontiguousarray(xt)

    QT = bucket_T(Q)
    KT = bucket_T(K)

    Vm = np.asarray(V).reshape(B, NB, BS, H, D) * valid[..., None, None]
    Vp = np.empty((B, NB, BS, H, D1), dtype=bf)
    Vp[..., :D] = Vm.astype(bf)
    Vp[..., D] = valid[..., None].astype(bf)

    in_maps = []
    for core in range(NCORES):
        b, part = divmod(core, CORES_PER_B)
        n0 = part * NB_LOC
        nsl = slice(n0, n0 + NB_LOC)
        in_maps.append(
            {
                "qt": QT[b, nsl],
                "kt": KT[b, nsl],
                "v": np.ascontiguousarray(Vp[b, nsl]).reshape(NB_LOC, BS, H * D1),
                "qbias": np.ascontiguousarray(qbias[b, nsl].T),
            }
        )
    return in_maps


def kernel(Q, K, V, scope_buckets, buck_size):
    from concourse.bass_utils import run_bass_kernel_spmd

    global _cached_nc
    assert int(buck_size) == BS
    assert Q.shape == (B, L, H, D)

    in_maps = _host_prep(Q, K, V, scope_buckets)
    if _cached_nc is None:
        _cached_nc = _build()
    res = run_bass_kernel_spmd(_cached_nc, in_maps, list(range(NCORES)))

    out = np.empty((B, L, H, D), dtype=np.float32)
    for core in range(NCORES):
        b, part = divmod(core, CORES_PER_B)
        n0 = part * NB_LOC
        sl = slice(n0 * BS, (n0 + NB_LOC) * BS)
        out[b, sl] = res.results[core]["o"].reshape(NB_LOC * BS, H, D)
    return out



# revision 4
# speedup vs baseline: 1.0836x; 1.0836x over previous
import numpy as np

B, L, H, D = 2, 65536, 8, 32
BS = 128
NB = L // BS
NCORES = 8
CORES_PER_B = NCORES // B
NB_LOC = NB // CORES_PER_B
CB = 8
NCHUNK = NB_LOC // CB
HD = H * D
D1 = D + 1
C = H * D1
SCALE = float(1.0 / np.sqrt(D))

NSCH = 2
SIGMA = 0.0579
ALPHA = np.float32(SCALE * np.log2(np.e) * 128.0)
BETA = np.float32(128.0 * (127.0 - SIGMA) + 0.5)

_cached_nc = None


def _build(num_devices=NCORES):
    import concourse.bass as bass
    import concourse.bacc as bacc
    import concourse.tile as tile
    from concourse import mybir
    from contextlib import ExitStack

    f32 = mybir.dt.float32
    bf16 = mybir.dt.bfloat16
    i16 = mybir.dt.int16

    nc = bacc.Bacc(
        "TRN2", target_bir_lowering=False, debug=False, num_devices=num_devices
    )
    QTd = nc.dram_tensor("qt", [BS, NB_LOC, HD], bf16, kind="ExternalInput").ap()
    KTd = nc.dram_tensor("kt", [BS, NB_LOC, HD], bf16, kind="ExternalInput").ap()
    Vd = nc.dram_tensor("v", [BS, NB_LOC, C], bf16, kind="ExternalInput").ap()
    Od = nc.dram_tensor("o", [BS, NB_LOC, C], bf16, kind="ExternalOutput").ap()

    with tile.TileContext(nc) as tc, ExitStack() as ctx:
        qk_pool = ctx.enter_context(tc.tile_pool(name="qk", bufs=4))
        v_pool = ctx.enter_context(tc.tile_pool(name="vp", bufs=4))
        out_pool = ctx.enter_context(tc.tile_pool(name="outp", bufs=4))
        ea_pool = ctx.enter_context(tc.tile_pool(name="ea", bufs=3))
        es_pool = ctx.enter_context(tc.tile_pool(name="es", bufs=3))
        ps_pool = ctx.enter_context(tc.tile_pool(name="ps", bufs=1, space="PSUM"))

        s_ps = ps_pool.tile([BS, 8, 512], f32)

        chunk_tiles = {}
        exp_tiles = {}

        def ensure_chunk(c):
            if c in chunk_tiles or c >= NCHUNK:
                return
            n0 = c * CB
            qt = qk_pool.tile([BS, CB, HD], bf16, tag="qt")
            kt = qk_pool.tile([BS, CB, HD], bf16, tag="kt")
            if c == 0:
                nc.sync.dma_start(out=qt[:, 0:2], in_=QTd[:, 0:2])
                nc.sync.dma_start(out=kt[:, 0:2], in_=KTd[:, 0:2])
                nc.sync.dma_start(out=qt[:, 2:CB], in_=QTd[:, 2:CB])
                nc.sync.dma_start(out=kt[:, 2:CB], in_=KTd[:, 2:CB])
            else:
                nc.sync.dma_start(out=qt, in_=QTd[:, n0 : n0 + CB])
                nc.sync.dma_start(out=kt, in_=KTd[:, n0 : n0 + CB])
            v_t = v_pool.tile([BS, CB, H, D1], bf16)
            nc.sync.dma_start(
                out=v_t, in_=Vd[:, n0 : n0 + CB].rearrange("p n (h e) -> p n h e", h=H)
            )
            o_sb = out_pool.tile([BS, CB, H, D1], bf16)
            chunk_tiles[c] = (qt, kt, v_t, o_sb)

        def emit_s(m):
            qt, kt, _, _ = chunk_tiles[m // CB]
            j = m % CB
            base = (m % 2) * 4
            for h in range(H):
                hh, r = divmod(h, 4)
                nc.tensor.matmul(
                    s_ps[:, base + r, hh * BS : (hh + 1) * BS],
                    kt[32 * r : 32 * (r + 1), j, hh * BS : (hh + 1) * BS],
                    qt[32 * r : 32 * (r + 1), j, hh * BS : (hh + 1) * BS],
                    start=True,
                    stop=True,
                    tile_position=(32 * r, 0),
                )

        def emit_x(m):
            base = (m % 2) * 4
            ea = ea_pool.tile([BS, 3, 2, BS], bf16, tag="ea")
            nc.scalar.activation(
                ea,
                s_ps[:, base + 1 : base + 4, 0 : 2 * BS].rearrange(
                    "p r (a q) -> p r a q", a=2
                ),
                mybir.ActivationFunctionType.Exp,
                scale=SCALE,
            )
            es = es_pool.tile([BS, 2, BS], i16, tag="es")
            nc.vector.tensor_scalar(
                es,
                s_ps[:, base, 0 : 2 * BS].rearrange("p (a q) -> p a q", a=2),
                float(ALPHA),
                float(BETA),
                op0=mybir.AluOpType.mult,
                op1=mybir.AluOpType.add,
            )
            exp_tiles[m] = (ea, es)

        def emit_pv(m):
            _, _, v_t, _ = chunk_tiles[m // CB]
            ea, es = exp_tiles.pop(m)
            es_bf = es[:, :, :].bitcast(bf16)
            j = m % CB
            base = (m % 2) * 4
            for h in (1, 2, 3, 5, 6, 7, 0, 4):
                hh, r = divmod(h, 4)
                stat = es_bf[:, hh] if r == 0 else ea[:, r - 1, hh]
                cb = base + (0 if h < 4 else 2)
                c0 = 2 * BS + (h % 4) * D1
                nc.tensor.matmul(
                    s_ps[:, cb, c0 : c0 + D1],
                    stat,
                    v_t[:, j, h],
                    start=True,
                    stop=True,
                )

        def emit_e(m):
            _, _, _, o_sb = chunk_tiles[m // CB]
            j = m % CB
            base = (m % 2) * 4
            corner4 = s_ps[:, base : base + 4, 2 * BS : 2 * BS + 4 * D1]
            src = bass.AP(
                tensor=corner4.tensor,
                offset=corner4.offset,
                ap=[corner4.ap[0], [corner4.ap[1][0] * 2, 2], corner4.ap[2]],
            )
            nc.vector.tensor_copy(
                o_sb[:, j].rearrange("p (b g) e -> p b (g e)", b=2),
                src,
            )
            if j == CB - 1:
                n0 = (m // CB) * CB
                nc.gpsimd.dma_start(
                    out=Od[:, n0 : n0 + CB],
                    in_=o_sb.rearrange("p c h e -> p c (h e)"),
                )

        ensure_chunk(0)
        ensure_chunk(1)
        for w in range(NB_LOC + 3):
            if w < NB_LOC:
                if w % CB == 0:
                    ensure_chunk(w // CB + 1)
                    ensure_chunk(w // CB + 2)
                emit_s(w)
            if 3 <= w <= NB_LOC + 2:
                emit_e(w - 3)
            if 1 <= w <= NB_LOC:
                emit_x(w - 1)
            if 2 <= w <= NB_LOC + 1:
                emit_pv(w - 2)

    nc.compile()
    return nc


def _host_prep(Q, K, V, scope_buckets):
    import ml_dtypes

    bf = ml_dtypes.bfloat16
    scope_buckets = np.asarray(scope_buckets)
    starts = scope_buckets[..., 0].astype(np.int64)
    ends = scope_buckets[..., 1].astype(np.int64)
    abs_pos = (np.arange(NB, dtype=np.int64) * BS)[:, None] + np.arange(BS)[None, :]
    valid = (abs_pos[None] >= starts[..., None]) & (abs_pos[None] < ends[..., None])
    valid = valid.astype(np.float32)

    def bucket_T(x):
        xb = np.asarray(x).astype(bf).reshape(B, NB, BS, 2, BS)
        xt = xb.transpose(0, 4, 1, 3, 2).reshape(B, BS, NB, HD)
        return xt

    QT = bucket_T(Q)
    KT = bucket_T(K)

    Vm = np.asarray(V).reshape(B, NB, BS, H, D) * valid[..., None, None]
    Vp = np.empty((B, NB, BS, H, D1), dtype=bf)
    Vp[..., :D] = Vm.astype(bf)
    Vp[..., D] = valid[..., None].astype(bf)
    Vp = Vp.transpose(0, 2, 1, 3, 4).reshape(B, BS, NB, C)

    in_maps = []
    for core in range(NCORES):
        b, part = divmod(core, CORES_PER_B)
        nsl = slice(part * NB_LOC, (part + 1) * NB_LOC)
        in_maps.append(
            {
                "qt": np.ascontiguousarray(QT[b, :, nsl]),
                "kt": np.ascontiguousarray(KT[b, :, nsl]),
                "v": np.ascontiguousarray(Vp[b, :, nsl]),
            }
        )
    return in_maps


def kernel(Q, K, V, scope_buckets, buck_size):
    from concourse.bass_utils import run_bass_kernel_spmd

    global _cached_nc
    assert int(buck_size) == BS
    assert Q.shape == (B, L, H, D)

    in_maps = _host_prep(Q, K, V, scope_buckets)
    if _cached_nc is None:
        _cached_nc = _build()
    res = run_bass_kernel_spmd(_cached_nc, in_maps, list(range(NCORES)))

    scope_buckets = np.asarray(scope_buckets)
    starts = scope_buckets[..., 0].astype(np.int64)
    ends = scope_buckets[..., 1].astype(np.int64)
    abs_pos = (np.arange(NB, dtype=np.int64) * BS)[:, None] + np.arange(BS)[None, :]
    valid = (abs_pos[None] >= starts[..., None]) & (abs_pos[None] < ends[..., None])

    out = np.empty((B, L, H, D), dtype=np.float32)
    for core in range(NCORES):
        b, part = divmod(core, CORES_PER_B)
        arr = res.results[core]["o"].astype(np.float32)
        arr = arr.reshape(BS, NB_LOC, H, D1)
        num = arr[..., :D]
        den = np.maximum(arr[..., D], 1e-30)[..., None]
        o = (num / den).transpose(1, 0, 2, 3)
        n0 = part * NB_LOC
        o *= valid[b, n0 : n0 + NB_LOC][..., None, None]
        out[b, n0 * BS : (n0 + NB_LOC) * BS] = o.reshape(NB_LOC * BS, H, D)
    return out


# revision 6
# speedup vs baseline: 1.0926x; 1.0083x over previous
import numpy as np

B, L, H, D = 2, 65536, 8, 32
BS = 128
NB = L // BS
NCORES = 8
CORES_PER_B = NCORES // B
NB_LOC = NB // CORES_PER_B
CB = 8
NCHUNK = NB_LOC // CB
HD = H * D
D1 = D + 1
C = H * D1
SCALE = float(1.0 / np.sqrt(D))

NSCH = 2
SIGMA = 0.0579
ALPHA = np.float32(SCALE * np.log2(np.e) * 128.0)
BETA = np.float32(128.0 * (127.0 - SIGMA) + 0.5)

_cached_nc = None


def _build(num_devices=NCORES):
    import concourse.bass as bass
    import concourse.bacc as bacc
    import concourse.tile as tile
    from concourse import mybir
    from contextlib import ExitStack

    f32 = mybir.dt.float32
    bf16 = mybir.dt.bfloat16
    i16 = mybir.dt.int16

    nc = bacc.Bacc(
        "TRN2", target_bir_lowering=False, debug=False, num_devices=num_devices
    )
    QTd = nc.dram_tensor("qt", [BS, NB_LOC, HD], bf16, kind="ExternalInput").ap()
    KTd = nc.dram_tensor("kt", [BS, NB_LOC, HD], bf16, kind="ExternalInput").ap()
    Vd = nc.dram_tensor("v", [BS, NB_LOC, C], bf16, kind="ExternalInput").ap()
    Od = nc.dram_tensor("o", [BS, NB_LOC, C], bf16, kind="ExternalOutput").ap()

    with tile.TileContext(nc) as tc, ExitStack() as ctx:
        qk_pool = ctx.enter_context(tc.tile_pool(name="qk", bufs=4))
        v_pool = ctx.enter_context(tc.tile_pool(name="vp", bufs=4))
        out_pool = ctx.enter_context(tc.tile_pool(name="outp", bufs=4))
        ea_pool = ctx.enter_context(tc.tile_pool(name="ea", bufs=3))
        es_pool = ctx.enter_context(tc.tile_pool(name="es", bufs=3))
        ps_pool = ctx.enter_context(tc.tile_pool(name="ps", bufs=1, space="PSUM"))

        s_ps = ps_pool.tile([BS, 8, 512], f32)

        chunk_tiles = {}
        exp_tiles = {}

        def ensure_chunk(c):
            if c in chunk_tiles or c >= NCHUNK:
                return
            n0 = c * CB
            qt = qk_pool.tile([BS, CB, HD], bf16, tag="qt")
            kt = qk_pool.tile([BS, CB, HD], bf16, tag="kt")
            if c == 0:
                nc.sync.dma_start(out=qt[:, 0:1], in_=QTd[:, 0:1])
                nc.sync.dma_start(out=kt[:, 0:1], in_=KTd[:, 0:1])
                nc.sync.dma_start(out=qt[:, 1:CB], in_=QTd[:, 1:CB])
                nc.sync.dma_start(out=kt[:, 1:CB], in_=KTd[:, 1:CB])
            else:
                nc.sync.dma_start(out=qt, in_=QTd[:, n0 : n0 + CB])
                nc.sync.dma_start(out=kt, in_=KTd[:, n0 : n0 + CB])
            v_t = v_pool.tile([BS, CB, H, D1], bf16)
            nc.sync.dma_start(
                out=v_t, in_=Vd[:, n0 : n0 + CB].rearrange("p n (h e) -> p n h e", h=H)
            )
            o_sb = out_pool.tile([BS, CB, H, D1], bf16)
            chunk_tiles[c] = (qt, kt, v_t, o_sb)

        def emit_s(m):
            qt, kt, _, _ = chunk_tiles[m // CB]
            j = m % CB
            base = (m % 2) * 4
            for h in range(H):
                hh, r = divmod(h, 4)
                nc.tensor.matmul(
                    s_ps[:, base + r, hh * BS : (hh + 1) * BS],
                    kt[32 * r : 32 * (r + 1), j, hh * BS : (hh + 1) * BS],
                    qt[32 * r : 32 * (r + 1), j, hh * BS : (hh + 1) * BS],
                    start=True,
                    stop=True,
                    tile_position=(32 * r, 0),
                )

        def emit_x(m):
            base = (m % 2) * 4
            ea = ea_pool.tile([BS, 3, 2, BS], bf16, tag="ea")
            nc.scalar.activation(
                ea,
                s_ps[:, base + 1 : base + 4, 0 : 2 * BS].rearrange(
                    "p r (a q) -> p r a q", a=2
                ),
                mybir.ActivationFunctionType.Exp,
                scale=SCALE,
            )
            es = es_pool.tile([BS, 2, BS], i16, tag="es")
            nc.vector.tensor_scalar(
                es,
                s_ps[:, base, 0 : 2 * BS].rearrange("p (a q) -> p a q", a=2),
                float(ALPHA),
                float(BETA),
                op0=mybir.AluOpType.mult,
                op1=mybir.AluOpType.add,
            )
            exp_tiles[m] = (ea, es)

        def emit_pv(m):
            _, _, v_t, _ = chunk_tiles[m // CB]
            ea, es = exp_tiles.pop(m)
            es_bf = es[:, :, :].bitcast(bf16)
            j = m % CB
            base = (m % 2) * 4
            for h in (1, 2, 3, 5, 6, 7, 0, 4):
                hh, r = divmod(h, 4)
                stat = es_bf[:, hh] if r == 0 else ea[:, r - 1, hh]
                cb = base + (0 if h < 4 else 2)
                c0 = 2 * BS + (h % 4) * D1
                nc.tensor.matmul(
                    s_ps[:, cb, c0 : c0 + D1],
                    stat,
                    v_t[:, j, h],
                    start=True,
                    stop=True,
                )

        def emit_e(m):
            _, _, _, o_sb = chunk_tiles[m // CB]
            j = m % CB
            base = (m % 2) * 4
            corner4 = s_ps[:, base : base + 4, 2 * BS : 2 * BS + 4 * D1]
            src = bass.AP(
                tensor=corner4.tensor,
                offset=corner4.offset,
                ap=[corner4.ap[0], [corner4.ap[1][0] * 2, 2], corner4.ap[2]],
            )
            nc.vector.tensor_copy(
                o_sb[:, j].rearrange("p (b g) e -> p b (g e)", b=2),
                src,
            )
            if j == CB - 1:
                n0 = (m // CB) * CB
                nc.gpsimd.dma_start(
                    out=Od[:, n0 : n0 + CB],
                    in_=o_sb.rearrange("p c h e -> p c (h e)"),
                )

        ensure_chunk(0)
        ensure_chunk(1)
        for w in range(NB_LOC + 3):
            if w < NB_LOC:
                if w % CB == 0:
                    ensure_chunk(w // CB + 1)
                    ensure_chunk(w // CB + 2)
                emit_s(w)
            if 1 <= w <= NB_LOC:
                emit_x(w - 1)
            if 2 <= w <= NB_LOC + 1:
                emit_pv(w - 2)
            if 3 <= w <= NB_LOC + 2:
                emit_e(w - 3)

    nc.compile()
    return nc


def _host_prep(Q, K, V, scope_buckets):
    import ml_dtypes

    bf = ml_dtypes.bfloat16
    scope_buckets = np.asarray(scope_buckets)
    starts = scope_buckets[..., 0].astype(np.int64)
    ends = scope_buckets[..., 1].astype(np.int64)
    abs_pos = (np.arange(NB, dtype=np.int64) * BS)[:, None] + np.arange(BS)[None, :]
    valid = (abs_pos[None] >= starts[..., None]) & (abs_pos[None] < ends[..., None])
    valid = valid.astype(np.float32)

    def bucket_T(x):
        xb = np.asarray(x).astype(bf).reshape(B, NB, BS, 2, BS)
        xt = xb.transpose(0, 4, 1, 3, 2).reshape(B, BS, NB, HD)
        return xt

    QT = bucket_T(Q)
    KT = bucket_T(K)

    Vm = np.asarray(V).reshape(B, NB, BS, H, D) * valid[..., None, None]
    Vp = np.empty((B, NB, BS, H, D1), dtype=bf)
    Vp[..., :D] = Vm.astype(bf)
    Vp[..., D] = valid[..., None].astype(bf)
    Vp = Vp.transpose(0, 2, 1, 3, 4).reshape(B, BS, NB, C)

    in_maps = []
    for core in range(NCORES):
        b, part = divmod(core, CORES_PER_B)
        nsl = slice(part * NB_LOC, (part + 1) * NB_LOC)
        in_maps.append(
            {
                "qt": np.ascontiguousarray(QT[b, :, nsl]),
                "kt": np.ascontiguousarray(KT[b, :, nsl]),
                "v": np.ascontiguousarray(Vp[b, :, nsl]),
            }
        )
    return in_maps


def kernel(Q, K, V, scope_buckets, buck_size):
    from concourse.bass_utils import run_bass_kernel_spmd

    global _cached_nc
    assert int(buck_size) == BS
    assert Q.shape == (B, L, H, D)

    in_maps = _host_prep(Q, K, V, scope_buckets)
    if _cached_nc is None:
        _cached_nc = _build()
    res = run_bass_kernel_spmd(_cached_nc, in_maps, list(range(NCORES)))

    scope_buckets = np.asarray(scope_buckets)
    starts = scope_buckets[..., 0].astype(np.int64)
    ends = scope_buckets[..., 1].astype(np.int64)
    abs_pos = (np.arange(NB, dtype=np.int64) * BS)[:, None] + np.arange(BS)[None, :]
    valid = (abs_pos[None] >= starts[..., None]) & (abs_pos[None] < ends[..., None])

    out = np.empty((B, L, H, D), dtype=np.float32)
    for core in range(NCORES):
        b, part = divmod(core, CORES_PER_B)
        arr = res.results[core]["o"].astype(np.float32)
        arr = arr.reshape(BS, NB_LOC, H, D1)
        num = arr[..., :D]
        den = np.maximum(arr[..., D], 1e-30)[..., None]
        o = (num / den).transpose(1, 0, 2, 3)
        n0 = part * NB_LOC
        o *= valid[b, n0 : n0 + NB_LOC][..., None, None]
        out[b, n0 * BS : (n0 + NB_LOC) * BS] = o.reshape(NB_LOC * BS, H, D)
    return out
